# revision 3
# baseline (speedup 1.0000x reference)
"""DeepSeek-MoE layer (shared SwiGLU expert + 8 routed GELU experts, top-2)
as a Bass/Tile kernel for 8 Trainium2 NeuronCores.

Sharding: data-parallel over tokens. Each core gets 512 of the 4096 tokens
(feature-major x slice) plus a replicated copy of all weights, computes
router + shared expert + all routed experts densely (combine weights are zero
for unselected experts), and writes its token-major output slice.

Matmuls run as float32r (tf32-like, full PE rate); the router matmul runs in
exact fp32 so top-k selection matches the fp32 reference.
"""
import sys
sys.path.insert(0, '/opt/trn_rl_repo')

import numpy as np
import concourse.bass as bass
import concourse.tile as tile
from concourse import mybir, bacc
from concourse.bass_utils import run_bass_kernel_spmd

N_CORES = 8
B, T = 2, 2048
D = 1024          # d_model
HS = 2048         # shared-expert hidden
HR = 1024         # routed-expert hidden
E = 8             # experts
NTOK = (B * T) // N_CORES   # tokens per core = 512
NCH = NTOK // 128           # token chunks of 128 = 4
KD = D // 128               # k-tiles over D = 8
KS = HS // 128              # k-tiles over HS = 16
KR = HR // 128              # k-tiles over HR = 8

F32 = mybir.dt.float32
F32R = mybir.dt.float32r
AF = mybir.ActivationFunctionType
ALU = mybir.AluOpType
AX = mybir.AxisListType

_CACHE = {}


def _build():
    nc = bacc.Bacc(None, target_bir_lowering=False)
    xt = nc.dram_tensor("xt", [D, NTOK], F32, kind="ExternalInput")
    rwt = nc.dram_tensor("rwt", [D, E], F32, kind="ExternalInput")
    bias = nc.dram_tensor("bias", [E], F32, kind="ExternalInput")
    sw1 = nc.dram_tensor("sw1", [D, HS], F32R, kind="ExternalInput")
    sw3 = nc.dram_tensor("sw3", [D, HS], F32R, kind="ExternalInput")
    sw2 = nc.dram_tensor("sw2", [HS, D], F32R, kind="ExternalInput")
    ew1 = nc.dram_tensor("ew1", [E, D, HR], F32R, kind="ExternalInput")
    ew2 = nc.dram_tensor("ew2", [E, HR, D], F32R, kind="ExternalInput")
    out = nc.dram_tensor("out", [NTOK, D], F32, kind="ExternalOutput")

    xr = xt.rearrange("(kt kp) n -> kp kt n", kp=128)            # [128, 8, 512]
    rwr = rwt.rearrange("(kt kp) e -> kp kt e", kp=128)          # [128, 8, 8]
    sw1r = sw1.rearrange("(kt kp) h -> kp kt h", kp=128)         # [128, 8, 2048]
    sw3r = sw3.rearrange("(kt kp) h -> kp kt h", kp=128)
    sw2r = sw2.rearrange("(kt kp) d -> kp kt d", kp=128)         # [128, 16, 1024]
    ew1r = ew1.rearrange("e (kt kp) h -> e kp kt h", kp=128)     # [8, 128, 8, 1024]
    ew2r = ew2.rearrange("e (kt kp) d -> e kp kt d", kp=128)
    outr = out.rearrange("(c p) d -> p c d", p=128)              # [128, 4, 1024]

    bias_bcast = bass.AP(tensor=bias, offset=0,
                         ap=[[0, 128], [1, E]])                  # replicate on parts

    with tile.TileContext(nc) as tc:
        with tc.tile_pool(name="persist", bufs=1) as persist, \
             tc.tile_pool(name="wstream", bufs=4) as wstream, \
             tc.tile_pool(name="hpool", bufs=1) as hpool, \
             tc.tile_pool(name="rpool", bufs=2) as rpool, \
             tc.tile_pool(name="small", bufs=1) as small, \
             tc.tile_pool(name="psA", bufs=3, space="PSUM") as psA, \
             tc.tile_pool(name="psY", bufs=4, space="PSUM") as psY, \
             tc.tile_pool(name="psR", bufs=1, space="PSUM") as psR:

            # ---- load x (fp32 for router) and a f32r copy for expert matmuls
            xf = persist.tile([128, KD, NTOK], F32)
            nc.sync.dma_start(out=xf, in_=xr)
            xq = persist.tile([128, KD, NTOK], F32R)
            for k in range(KD):
                nc.vector.tensor_copy(xq[:, k, :], xf[:, k, :])

            # ---- router: scores token-major [128, E] per chunk, exact fp32
            rw_sb = small.tile([128, KD, E], F32)
            nc.sync.dma_start(out=rw_sb, in_=rwr)
            bias_sb = small.tile([128, E], F32)
            nc.gpsimd.dma_start(out=bias_sb, in_=bias_bcast)
            comb = persist.tile([128, NCH, E], F32)
            for c in range(NCH):
                pr = psR.tile([128, E], F32, tag="pr")
                for k in range(KD):
                    nc.tensor.matmul(pr[:, :], xf[:, k, c * 128:(c + 1) * 128],
                                     rw_sb[:, k, :],
                                     start=(k == 0), stop=(k == KD - 1))
                s = rpool.tile([128, E], F32, tag="s")
                nc.scalar.activation(s[:, :], pr[:, :], AF.Sigmoid)
                selp = rpool.tile([128, E], F32, tag="selp")
                nc.vector.tensor_add(selp[:, :], s[:, :], bias_sb[:, :])
                m1 = rpool.tile([128, 1], F32, tag="m1")
                nc.vector.reduce_max(m1[:, :], selp[:, :], axis=AX.X)
                eq = rpool.tile([128, E], F32, tag="eq")
                nc.vector.tensor_scalar(eq[:, :], selp[:, :], m1[:, :], None,
                                        op0=ALU.is_ge)
                nc.vector.tensor_scalar_mul(eq[:, :], eq[:, :], -1e30)
                nc.vector.tensor_add(eq[:, :], selp[:, :], eq[:, :])
                m2 = rpool.tile([128, 1], F32, tag="m2")
                nc.vector.reduce_max(m2[:, :], eq[:, :], axis=AX.X)
                mask2 = rpool.tile([128, E], F32, tag="mask2")
                nc.vector.tensor_scalar(mask2[:, :], selp[:, :], m2[:, :], None,
                                        op0=ALU.is_ge)
                gun = rpool.tile([128, E], F32, tag="gun")
                nc.vector.tensor_mul(gun[:, :], s[:, :], mask2[:, :])
                den = rpool.tile([128, 1], F32, tag="den")
                nc.vector.reduce_sum(den[:, :], gun[:, :], axis=AX.X)
                nc.vector.tensor_scalar_add(den[:, :], den[:, :], 1e-9)
                dinv = rpool.tile([128, 1], F32, tag="dinv")
                nc.vector.reciprocal(dinv[:, :], den[:, :])
                nc.vector.tensor_scalar(comb[:, c, :], gun[:, :], dinv[:, :], None,
                                        op0=ALU.mult)

            # ---- shared expert stage 1: P = silu(x@sw1) * (x@sw3), f-major
            pshr = persist.tile([128, KS, NTOK], F32R)   # P^T [2048, 512]
            for p in range(4):                            # h-col pieces of 512
                w1p = wstream.tile([128, KD, 512], F32R, tag="w")
                nc.sync.dma_start(out=w1p, in_=sw1r[:, :, p * 512:(p + 1) * 512])
                w3p = wstream.tile([128, KD, 512], F32R, tag="w")
                nc.sync.dma_start(out=w3p, in_=sw3r[:, :, p * 512:(p + 1) * 512])
                for m in range(4):                        # h2-tiles inside piece
                    h2 = p * 4 + m
                    pa = psA.tile([128, NTOK], F32, tag="pa")
                    for k in range(KD):
                        nc.tensor.matmul(pa[:, :], w1p[:, k, m * 128:(m + 1) * 128],
                                         xq[:, k, :], start=(k == 0), stop=(k == KD - 1))
                    pg = psA.tile([128, NTOK], F32, tag="pa")
                    for k in range(KD):
                        nc.tensor.matmul(pg[:, :], w3p[:, k, m * 128:(m + 1) * 128],
                                         xq[:, k, :], start=(k == 0), stop=(k == KD - 1))
                    asb = rpool.tile([128, NTOK], F32, tag="asb")
                    nc.scalar.activation(asb[:, :], pa[:, :], AF.Silu)
                    nc.vector.tensor_mul(pshr[:, h2, :], asb[:, :], pg[:, :])

            # ---- shared expert stage 2: acc = P @ sw2, token-major
            acc = persist.tile([128, NCH, D], F32)
            for dh in range(2):
                pys = [psY.tile([128, 512], F32, tag="py", name=f"py_sh{dh}{c}")
                       for c in range(NCH)]
                for kh in range(2):
                    w2p = wstream.tile([128, KD, 512], F32R, tag="w")
                    nc.sync.dma_start(
                        out=w2p,
                        in_=sw2r[:, kh * 8:(kh + 1) * 8, dh * 512:(dh + 1) * 512])
                    for c in range(NCH):
                        for k in range(KD):
                            nc.tensor.matmul(
                                pys[c][:, :],
                                pshr[:, kh * 8 + k, c * 128:(c + 1) * 128],
                                w2p[:, k, :],
                                start=(kh == 0 and k == 0),
                                stop=(kh == 1 and k == KD - 1))
                for c in range(NCH):
                    nc.vector.tensor_copy(
                        acc[:, c, dh * 512:(dh + 1) * 512], pys[c][:, :])

            # ---- routed experts (dense over experts; comb zeroes non-selected)
            for e in range(E):
                ht = hpool.tile([128, KR, NTOK], F32R, tag="h")   # gelu(x@ew1[e])^T
                for hh in range(2):
                    w1p = wstream.tile([128, KD, 512], F32R, tag="w")
                    nc.sync.dma_start(out=w1p,
                                      in_=ew1r[e, :, :, hh * 512:(hh + 1) * 512])
                    for m in range(4):
                        pa = psA.tile([128, NTOK], F32, tag="pa")
                        for k in range(KD):
                            nc.tensor.matmul(pa[:, :],
                                             w1p[:, k, m * 128:(m + 1) * 128],
                                             xq[:, k, :],
                                             start=(k == 0), stop=(k == KD - 1))
                        nc.scalar.activation(ht[:, hh * 4 + m, :], pa[:, :], AF.Gelu)
                for dh in range(2):
                    w2p = wstream.tile([128, KR, 512], F32R, tag="w")
                    nc.sync.dma_start(out=w2p,
                                      in_=ew2r[e, :, :, dh * 512:(dh + 1) * 512])
                    for c in range(NCH):
                        py = psY.tile([128, 512], F32, tag="py")
                        for k in range(KR):
                            nc.tensor.matmul(py[:, :],
                                             ht[:, k, c * 128:(c + 1) * 128],
                                             w2p[:, k, :],
                                             start=(k == 0), stop=(k == KR - 1))
                        # acc += comb[:, e] * y
                        nc.vector.scalar_tensor_tensor(
                            acc[:, c, dh * 512:(dh + 1) * 512],
                            py[:, :], comb[:, c, e:e + 1],
                            acc[:, c, dh * 512:(dh + 1) * 512],
                            op0=ALU.mult, op1=ALU.add)

            # ---- store token-major output
            nc.sync.dma_start(out=outr, in_=acc)
    nc.compile()
    return nc


def _get_nc():
    if "nc" not in _CACHE:
        _CACHE["nc"] = _build()
    return _CACHE["nc"]


def kernel(x, router_w, router_bias, sw1, sw3, sw2, ew1, ew2):
    nc = _get_nc()
    xf = np.ascontiguousarray(x, dtype=np.float32).reshape(B * T, D)
    rwt = np.ascontiguousarray(router_w.T, dtype=np.float32)
    bias = np.ascontiguousarray(router_bias, dtype=np.float32)
    sw1 = np.ascontiguousarray(sw1, dtype=np.float32)
    sw3 = np.ascontiguousarray(sw3, dtype=np.float32)
    sw2 = np.ascontiguousarray(sw2, dtype=np.float32)
    ew1 = np.ascontiguousarray(ew1, dtype=np.float32)
    ew2 = np.ascontiguousarray(ew2, dtype=np.float32)

    in_maps = []
    for c in range(N_CORES):
        xsl = xf[c * NTOK:(c + 1) * NTOK]                 # [512, 1024]
        in_maps.append({
            "xt": np.ascontiguousarray(xsl.T),            # [1024, 512]
            "rwt": rwt, "bias": bias,
            "sw1": sw1, "sw3": sw3, "sw2": sw2,
            "ew1": ew1, "ew2": ew2,
        })
    res = run_bass_kernel_spmd(nc, in_maps, core_ids=list(range(N_CORES)))
    outs = [res.results[c]["out"] for c in range(N_CORES)]
    return np.concatenate(outs, axis=0).reshape(B, T, D).astype(np.float32)


# revision 12
# speedup vs baseline: 1.0144x; 1.0144x over previous
"""DeepSeek-MoE layer (shared SwiGLU expert + 8 routed GELU experts, top-2)
as a Bass/Tile kernel for 8 Trainium2 NeuronCores.

Sharding: data-parallel over tokens. Each core gets 512 of the 4096 tokens
(feature-major x slice) plus a replicated copy of all weights, computes
router + shared expert + all routed experts densely (combine weights are zero
for unselected experts), and writes its token-major output slice.

Matmuls run as float32r (tf32-like, full PE rate); the router matmul runs in
exact fp32 so top-k selection matches the fp32 reference.
"""
import sys
sys.path.insert(0, '/opt/trn_rl_repo')

import numpy as np
import concourse.bass as bass
import concourse.tile as tile
from concourse import mybir, bacc
from concourse.bass_utils import run_bass_kernel_spmd

N_CORES = 8
B, T = 2, 2048
D = 1024          # d_model
HS = 2048         # shared-expert hidden
HR = 1024         # routed-expert hidden
E = 8             # experts
NTOK = (B * T) // N_CORES   # tokens per core = 512
NCH = NTOK // 128           # token chunks of 128 = 4
KD = D // 128               # k-tiles over D = 8
KS = HS // 128               # k-tiles over HS = 16
KR = HR // 128              # k-tiles over HR = 8

F32 = mybir.dt.float32
F32R = mybir.dt.float32r
AF = mybir.ActivationFunctionType
ALU = mybir.AluOpType
AX = mybir.AxisListType

_CACHE = {}


def _build():
    nc = bacc.Bacc(None, target_bir_lowering=False)
    xt = nc.dram_tensor("xt", [D, NTOK], F32, kind="ExternalInput")
    rwt = nc.dram_tensor("rwt", [D, E], F32, kind="ExternalInput")
    bias = nc.dram_tensor("bias", [E], F32, kind="ExternalInput")
    sw1 = nc.dram_tensor("sw1", [D, HS], F32R, kind="ExternalInput")
    sw3 = nc.dram_tensor("sw3", [D, HS], F32R, kind="ExternalInput")
    sw2 = nc.dram_tensor("sw2", [HS, D], F32R, kind="ExternalInput")
    ew1 = nc.dram_tensor("ew1", [E, D, HR], F32R, kind="ExternalInput")
    ew2 = nc.dram_tensor("ew2", [E, HR, D], F32R, kind="ExternalInput")
    out = nc.dram_tensor("out", [NTOK, D], F32, kind="ExternalOutput")

    xr = xt.rearrange("(kt kp) n -> kp kt n", kp=128)            # [128, 8, 512]
    rwr = rwt.rearrange("(kt kp) e -> kp kt e", kp=128)          # [128, 8, 8]
    sw1r = sw1.rearrange("(kt kp) h -> kp kt h", kp=128)         # [128, 8, 2048]
    sw3r = sw3.rearrange("(kt kp) h -> kp kt h", kp=128)
    sw2r = sw2.rearrange("(kt kp) d -> kp kt d", kp=128)         # [128, 16, 1024]
    ew1r = ew1.rearrange("e (kt kp) h -> e kp kt h", kp=128)     # [8, 128, 8, 1024]
    ew2r = ew2.rearrange("e (kt kp) d -> e kp kt d", kp=128)
    outr = out.rearrange("(c p) d -> p c d", p=128)              # [128, 4, 1024]

    bias_bcast = bass.AP(tensor=bias, offset=0,
                         ap=[[0, 128], [1, E]])                  # replicate on parts

    with tile.TileContext(nc) as tc:
        with tc.tile_pool(name="persist", bufs=1) as persist, \
             tc.tile_pool(name="wstream", bufs=4) as wstream, \
             tc.tile_pool(name="hpool", bufs=2) as hpool, \
             tc.tile_pool(name="rpool", bufs=2) as rpool, \
             tc.tile_pool(name="small", bufs=1) as small, \
             tc.tile_pool(name="psA", bufs=3, space="PSUM") as psA, \
             tc.tile_pool(name="psY", bufs=4, space="PSUM") as psY, \
             tc.tile_pool(name="psR", bufs=1, space="PSUM") as psR:

            # ---- PE warm-up burst: drives HAM to K=8/8 while DMAs land
            wuf = small.tile([128, 512], F32)
            nc.vector.memset(wuf[:, :], 1.0)
            wu = small.tile([128, 512], F32R)
            nc.vector.tensor_copy(wu[:, :], wuf[:, :])
            pwu = psY.tile([128, 512], F32, tag="py")
            for i in range(48):
                nc.tensor.matmul(pwu[:, :], wu[:, 0:128], wu[:, :],
                                 start=(i == 0), stop=(i == 47))

            # ---- load x (fp32 for router) per k-tile; f32r copy for experts
            xf = persist.tile([128, KD, NTOK], F32)
            xq = persist.tile([128, KD, NTOK], F32R)
            for k in range(KD):
                nc.sync.dma_start(out=xf[:, k, :], in_=xr[:, k, :])
                nc.vector.tensor_copy(xq[:, k, :], xf[:, k, :])

            # ---- router: scores token-major [128, E] per chunk, exact fp32
            rw_sb = small.tile([128, KD, E], F32)
            nc.sync.dma_start(out=rw_sb, in_=rwr)
            bias_sb = small.tile([128, E], F32)
            nc.gpsimd.dma_start(out=bias_sb, in_=bias_bcast)
            comb = persist.tile([128, NCH, E], F32)
            for c in range(NCH):
                pr = psR.tile([128, E], F32, tag="pr")
                for k in range(KD):
                    nc.tensor.matmul(pr[:, :], xf[:, k, c * 128:(c + 1) * 128],
                                     rw_sb[:, k, :],
                                     start=(k == 0), stop=(k == KD - 1))
                s = rpool.tile([128, E], F32, tag="s")
                nc.scalar.activation(s[:, :], pr[:, :], AF.Sigmoid)
                selp = rpool.tile([128, E], F32, tag="selp")
                nc.vector.tensor_add(selp[:, :], s[:, :], bias_sb[:, :])
                m1 = rpool.tile([128, 1], F32, tag="m1")
                nc.vector.reduce_max(m1[:, :], selp[:, :], axis=AX.X)
                eq = rpool.tile([128, E], F32, tag="eq")
                nc.vector.tensor_scalar(eq[:, :], selp[:, :], m1[:, :], None,
                                        op0=ALU.is_ge)
                nc.vector.tensor_scalar_mul(eq[:, :], eq[:, :], -1e30)
                nc.vector.tensor_add(eq[:, :], selp[:, :], eq[:, :])
                m2 = rpool.tile([128, 1], F32, tag="m2")
                nc.vector.reduce_max(m2[:, :], eq[:, :], axis=AX.X)
                mask2 = rpool.tile([128, E], F32, tag="mask2")
                nc.vector.tensor_scalar(mask2[:, :], selp[:, :], m2[:, :], None,
                                        op0=ALU.is_ge)
                gun = rpool.tile([128, E], F32, tag="gun")
                nc.vector.tensor_mul(gun[:, :], s[:, :], mask2[:, :])
                den = rpool.tile([128, 1], F32, tag="den")
                nc.vector.reduce_sum(den[:, :], gun[:, :], axis=AX.X)
                nc.vector.tensor_scalar_add(den[:, :], den[:, :], 1e-9)
                dinv = rpool.tile([128, 1], F32, tag="dinv")
                nc.vector.reciprocal(dinv[:, :], den[:, :])
                nc.vector.tensor_scalar(comb[:, c, :], gun[:, :], dinv[:, :], None,
                                        op0=ALU.mult)

            # ---- shared expert stage 1: P = silu(x@sw1) * (x@sw3), f-major
            pshr = persist.tile([128, KS, NTOK], F32R)   # P^T [2048, 512]
            for p in range(4):                            # h-col pieces of 512
                w1p = wstream.tile([128, KD, 512], F32R, tag="w", name=f"w1p{p}")
                nc.sync.dma_start(out=w1p, in_=sw1r[:, :, p * 512:(p + 1) * 512])
                w3p = wstream.tile([128, KD, 512], F32R, tag="w", name=f"w3p{p}")
                nc.sync.dma_start(out=w3p, in_=sw3r[:, :, p * 512:(p + 1) * 512])
                for m in range(4):                        # h2-tiles inside piece
                    h2 = p * 4 + m
                    pa = psA.tile([128, NTOK], F32, tag="pa")
                    for k in range(KD):
                        nc.tensor.matmul(pa[:, :], w1p[:, k, m * 128:(m + 1) * 128],
                                         xq[:, k, :], start=(k == 0), stop=(k == KD - 1))
                    pg = psA.tile([128, NTOK], F32, tag="pa")
                    for k in range(KD):
                        nc.tensor.matmul(pg[:, :], w3p[:, k, m * 128:(m + 1) * 128],
                                         xq[:, k, :], start=(k == 0), stop=(k == KD - 1))
                    asb = rpool.tile([128, NTOK], F32, tag="asb")
                    nc.scalar.activation(asb[:, :], pa[:, :], AF.Silu)
                    nc.vector.tensor_mul(pshr[:, h2, :], asb[:, :], pg[:, :])

            # ---- shared expert stage 2: acc = P @ sw2, token-major
            # (4 PSUM banks live per d-half; kh pieces streamed sequentially)
            acc = persist.tile([128, NCH, D], F32)
            for dh in range(2):
                pys = [psY.tile([128, 512], F32, tag="py", name=f"py_sh{dh}{c}")
                       for c in range(NCH)]
                for kh in range(2):                       # kt halves of HS
                    w2p = wstream.tile([128, KD, 512], F32R, tag="w",
                                       name=f"w2p{dh}{kh}")
                    nc.sync.dma_start(
                        out=w2p,
                        in_=sw2r[:, kh * 8:(kh + 1) * 8, dh * 512:(dh + 1) * 512])
                    for c in range(NCH):
                        for k in range(KD):
                            kk = kh * 8 + k
                            nc.tensor.matmul(
                                pys[c][:, :],
                                pshr[:, kk, c * 128:(c + 1) * 128],
                                w2p[:, k, :],
                                start=(kk == 0), stop=(kk == KS - 1))
                for c in range(NCH):
                    nc.vector.tensor_copy(acc[:, c, dh * 512:(dh + 1) * 512],
                                          pys[c][:, :])

            # ---- routed experts (dense over experts; comb zeroes non-selected)
            for e in range(E):
                ht = hpool.tile([128, KR, NTOK], F32R, tag="h")   # gelu(x@ew1[e])^T
                for hh in range(2):
                    w1e = wstream.tile([128, KD, 512], F32R, tag="w",
                                       name=f"ew1p{e}{hh}")
                    nc.sync.dma_start(out=w1e,
                                      in_=ew1r[e][:, :, hh * 512:(hh + 1) * 512])
                    for m in range(4):
                        pa = psA.tile([128, NTOK], F32, tag="pa")
                        for k in range(KD):
                            nc.tensor.matmul(pa[:, :],
                                             w1e[:, k, m * 128:(m + 1) * 128],
                                             xq[:, k, :],
                                             start=(k == 0), stop=(k == KD - 1))
                        nc.scalar.activation(ht[:, hh * 4 + m, :], pa[:, :], AF.Gelu)
                w2e = [None, None]
                for dh in range(2):
                    w2e[dh] = wstream.tile([128, KR, 512], F32R, tag="w",
                                           name=f"ew2p{e}{dh}")
                    nc.sync.dma_start(out=w2e[dh],
                                      in_=ew2r[e][:, :, dh * 512:(dh + 1) * 512])
                for c in range(NCH):
                    py = psY.tile([128, 512], F32, tag="py")
                    py2 = psY.tile([128, 512], F32, tag="py")
                    for k in range(KR):
                        nc.tensor.matmul(py[:, :],
                                         ht[:, k, c * 128:(c + 1) * 128],
                                         w2e[0][:, k, :],
                                         start=(k == 0), stop=(k == KR - 1))
                        nc.tensor.matmul(py2[:, :],
                                         ht[:, k, c * 128:(c + 1) * 128],
                                         w2e[1][:, k, :],
                                         start=(k == 0), stop=(k == KR - 1))
                    # acc += comb[:, e] * y
                    nc.vector.scalar_tensor_tensor(
                        acc[:, c, 0:512],
                        py[:, :], comb[:, c, e:e + 1],
                        acc[:, c, 0:512],
                        op0=ALU.mult, op1=ALU.add)
                    nc.vector.scalar_tensor_tensor(
                        acc[:, c, 512:1024],
                        py2[:, :], comb[:, c, e:e + 1],
                        acc[:, c, 512:1024],
                        op0=ALU.mult, op1=ALU.add)
                    if e == E - 1:
                        nc.sync.dma_start(out=outr[:, c, :], in_=acc[:, c, :])
    nc.compile()
    return nc


def _get_nc():
    if "nc" not in _CACHE:
        _CACHE["nc"] = _build()
    return _CACHE["nc"]


def _make_in_maps(inputs):
    x = inputs["x"]
    xf = np.ascontiguousarray(x, dtype=np.float32).reshape(B * T, D)
    rwt = np.ascontiguousarray(np.asarray(inputs["router_w"]).T, dtype=np.float32)
    bias = np.ascontiguousarray(inputs["router_bias"], dtype=np.float32)
    sw1 = np.ascontiguousarray(inputs["sw1"], dtype=np.float32)
    sw3 = np.ascontiguousarray(inputs["sw3"], dtype=np.float32)
    sw2 = np.ascontiguousarray(inputs["sw2"], dtype=np.float32)
    ew1 = np.ascontiguousarray(inputs["ew1"], dtype=np.float32)
    ew2 = np.ascontiguousarray(inputs["ew2"], dtype=np.float32)
    in_maps = []
    for c in range(N_CORES):
        xsl = xf[c * NTOK:(c + 1) * NTOK]                 # [512, 1024]
        in_maps.append({
            "xt": np.ascontiguousarray(xsl.T),            # [1024, 512]
            "rwt": rwt, "bias": bias,
            "sw1": sw1, "sw3": sw3, "sw2": sw2,
            "ew1": ew1, "ew2": ew2,
        })
    return in_maps


def kernel(x, router_w, router_bias, sw1, sw3, sw2, ew1, ew2):
    nc = _get_nc()
    in_maps = _make_in_maps(dict(x=x, router_w=router_w, router_bias=router_bias,
                                 sw1=sw1, sw3=sw3, sw2=sw2, ew1=ew1, ew2=ew2))
    res = run_bass_kernel_spmd(nc, in_maps, core_ids=list(range(N_CORES)))
    outs = [res.results[c]["out"] for c in range(N_CORES)]
    return np.concatenate(outs, axis=0).reshape(B, T, D).astype(np.float32)
